# revision 1
# baseline (speedup 1.0000x reference)
"""Trainium2 Bass kernel for a dense transformer block (self-attn + cross-attn + MLP).

Sharding: data-parallel over batch, one batch element per NeuronCore (B=8, 8 cores),
no collectives. All activations are kept feature-major ([C, T]) on chip so every
projection matmul consumes weight tiles directly from DRAM (lhsT = W[k,m] slabs) and
activations as the moving operand; matmuls run in float32r (full PE rate at moving
dim >= 256, ~1e-4 relative rounding).

Self/cross attention uses the S^T ([keys, queries]) layout: softmax reduction over
keys is the PSUM accumulation direction; the denominator comes free from a ones
column appended to V (row 64 of the AV matmul output); 1/den is broadcast across
partitions with a K=1 ones-matmul on the PE. Causal masking multiplies exp(S^T)
diagonal tiles by slices of one precomputed [128, 896] master mask.

SBUF pools are stack-allocated per side; long-lived tensors (xT, v_aug, q/k, the
residual, u) live on the right-side stack, phase transients on the left.
"""

import sys
import numpy as np

sys.path.insert(0, "/opt/trn_rl_repo")

B, T, C = 8, 1024, 1024
H = 16
D = C // H          # 64
TI = 256
FF = 4 * C          # 4096
EPS = 1e-5
NCT = C // 128      # 8 c tiles
NTT = T // 128      # 8 t tiles
NFT = FF // 128     # 32 ff tiles
P = 128

_CACHED = {}


def _build():
    import concourse.tile as tile
    from concourse import bacc, mybir
    from concourse.masks import make_identity

    F32, F32R = mybir.dt.float32, mybir.dt.float32r
    AF = mybir.ActivationFunctionType
    OP = mybir.AluOpType

    nc = bacc.Bacc("TRN2", target_bir_lowering=False, debug=False, num_devices=8)

    dr = {}
    dr["x"] = nc.dram_tensor("x", [T, C], F32, kind="ExternalInput")
    dr["x_img_feats"] = nc.dram_tensor("x_img_feats", [TI, C], F32, kind="ExternalInput")
    for nm, shp in [
        ("ln1_g", [C]), ("ln1_b", [C]), ("ln2_g", [C]), ("ln2_b", [C]),
        ("W_attn", [C, 3 * C]), ("b_attn", [3 * C]),
        ("W_aproj", [C, C]), ("b_aproj", [C]),
        ("Wq", [C, C]), ("bq", [C]), ("Wk", [C, C]), ("bk", [C]),
        ("Wv", [C, C]), ("bv", [C]), ("Wcproj", [C, C]), ("bcproj", [C]),
        ("W_fc", [C, FF]), ("b_fc", [FF]), ("W_mproj", [FF, C]), ("b_mproj", [C]),
    ]:
        dr[nm] = nc.dram_tensor(nm, shp, F32, kind="ExternalInput")
    out_d = nc.dram_tensor("out", [T, C], F32, kind="ExternalOutput")

    def W2d(name):
        return dr[name].ap()

    with tile.TileContext(nc) as tc, nc.allow_low_precision(
        reason="float32r rounding of matmul operands is intentional"
    ):
        kw_cms = []

        def openp(**kw):
            cm = tc.tile_pool(**kw)
            return cm, cm.__enter__()

        def openkw(**kw):
            cm, p = openp(**kw)
            kw_cms.append(cm)
            return p

        # ---------------- kernel-wide pools (left-stack base) ----------------
        constp = openkw(name="const", bufs=1)
        scrp = openkw(name="scr", bufs=2)       # f32 [128,1024] ln scratch
        fsrp = openkw(name="fsr", bufs=2)       # f32r [128,512] squares
        abp = openkw(name="ab", bufs=1)         # A_b/B_b [128,1024]
        rowp = openkw(name="rows", bufs=5)      # one [1,1024] "row" tag
        rbp = openkw(name="rb", bufs=3)         # [64,512] + [1,512] rden
        dramp = openkw(name="dram", bufs=1, space="DRAM")

        # ---------------- constants ----------------
        ident = constp.tile([P, P], F32)
        make_identity(nc, ident)
        identR = constp.tile([P, P], F32R)
        nc.vector.tensor_copy(out=identR, in_=ident)

        ones_col = constp.tile([P, 16], F32)
        nc.vector.memset(ones_col, 1.0)
        ones128R = constp.tile([P, 1], F32R)
        nc.vector.tensor_copy(out=ones128R, in_=ones_col[:, 0:1])
        o1x = constp.tile([1, P], F32)
        nc.vector.memset(o1x, 1.0)
        ones_1x128 = constp.tile([1, P], F32R)
        nc.vector.tensor_copy(out=ones_1x128, in_=o1x)
        eps_t = constp.tile([1, 1], F32)
        nc.vector.memset(eps_t, EPS)
        zeros384 = constp.tile([P, 384], F32)
        nc.vector.memset(zeros384, 0.0)

        # master causal mask [128, 896]: keep (1.0) iff (col - row - 384) >= 0.
        # slice [:, 384-128j : 896-128j] == keep iff t_local >= s_local + 128*j
        master = constp.tile([P, 896], F32)
        nc.gpsimd.memset(master, 1.0)
        nc.gpsimd.affine_select(
            out=master, in_=master, compare_op=OP.is_ge, fill=0.0,
            base=-384, pattern=[[1, 896]], channel_multiplier=-1)

        def load_cols(name, nf):
            t = constp.tile([P, nf], F32, name=name + "_c")
            nc.sync.dma_start(out=t, in_=dr[name].ap().rearrange("(f p) -> p f", p=P))
            return t

        g1, b1 = load_cols("ln1_g", NCT), load_cols("ln1_b", NCT)
        g2, b2 = load_cols("ln2_g", NCT), load_cols("ln2_b", NCT)
        bqk = constp.tile([P, 16], F32)
        nc.sync.dma_start(out=bqk, in_=dr["b_attn"].ap()[0:2 * C].rearrange("(f p) -> p f", p=P))
        bq_c = load_cols("bq", NCT)
        bk_c = load_cols("bk", NCT)
        bap_c = load_cols("b_aproj", NCT)
        bcp_c = load_cols("bcproj", NCT)
        bmp_c = load_cols("b_mproj", NCT)
        bfc_c = load_cols("b_fc", NFT)

        xT_d = dramp.tile([NCT, P, T], F32R)    # residual spill

        # ---------------- helpers ----------------
        def bcast_row(row_f32, dest_pool, psp, tag):
            """[1, C] f32 row -> [128, C] f32 partition-broadcast tile."""
            rowr = rowp.tile([1, C], F32R, tag="row", name="rowr")
            nc.vector.tensor_copy(out=rowr, in_=row_f32)
            dest = dest_pool.tile([P, C], F32, tag=tag, name=tag)
            for cc in range(2):
                bps = psp.tile([P, 512], F32, tag="bc", name="bc")
                nc.tensor.matmul(bps, ones_1x128, rowr[:, 512 * cc:512 * (cc + 1)],
                                 start=True, stop=True)
                nc.scalar.copy(out=dest[:, 512 * cc:512 * (cc + 1)], in_=bps)
            return dest

        def load_wslab(wap, co, wpool, eng=None):
            """W[:, co*128:(co+1)*128] ([K, 128]) -> f32r [128, K/128, 128]."""
            nk = wap.shape[0] // P
            stage = wpool.tile([P, nk, P], F32, tag="ws", name="ws")
            nc.sync.dma_start(
                out=stage,
                in_=wap[:, co * P:(co + 1) * P].rearrange("(c p) f -> p c f", p=P))
            wr = wpool.tile([P, nk, P], F32R, tag="wr", name="wr")
            (eng or nc.gpsimd).tensor_copy(out=wr, in_=stage)
            return wr

        def ln_stats(xtiles, psp):
            """Feature-dim LN stats for feature-major tiles -> (A_b, B_b) [128,T]
            f32 broadcast tiles with xhat = x*A_b + B_b (A=rstd, B=-mu*rstd)."""
            sum_ps, sq_ps = [], []
            for tch in range(2):
                sp = psp.tile([1, 512], F32, tag="lnsum", name="lnsum")
                qp = psp.tile([1, 512], F32, tag="lnsq", name="lnsq")
                for c in range(NCT):
                    xs = xtiles[c][:, 512 * tch:512 * (tch + 1)]
                    nc.tensor.matmul(sp, ones128R, xs, start=(c == 0), stop=(c == NCT - 1))
                    sq = fsrp.tile([P, 512], F32R, tag="sq", name="sq")
                    nc.vector.tensor_tensor(out=sq, in0=xs, in1=xs, op=OP.mult)
                    nc.tensor.matmul(qp, ones128R, sq, start=(c == 0), stop=(c == NCT - 1))
                sum_ps.append(sp)
                sq_ps.append(qp)
            mu = rowp.tile([1, T], F32, tag="row", name="mu")
            msq = rowp.tile([1, T], F32, tag="row", name="msq")
            for tch in range(2):
                sl = slice(512 * tch, 512 * (tch + 1))
                nc.vector.tensor_scalar_mul(out=mu[:, sl], in0=sum_ps[tch], scalar1=1.0 / C)
                nc.vector.tensor_scalar_mul(out=msq[:, sl], in0=sq_ps[tch], scalar1=1.0 / C)
            musq = rowp.tile([1, T], F32, tag="row", name="musq")
            nc.vector.tensor_tensor(out=musq, in0=mu, in1=mu, op=OP.mult)
            nc.vector.tensor_tensor(out=msq, in0=msq, in1=musq, op=OP.subtract)
            nc.scalar.activation(out=musq, in_=msq, func=AF.Sqrt, bias=eps_t, scale=1.0)
            arow = rowp.tile([1, T], F32R, tag="row", name="arow")
            nc.vector.reciprocal(out=arow, in_=musq)
            brow = rowp.tile([1, T], F32R, tag="row", name="brow")
            nc.vector.scalar_tensor_tensor(out=brow, in0=mu, scalar=-1.0, in1=arow,
                                           op0=OP.mult, op1=OP.mult)
            A_b = abp.tile([P, T], F32, tag="A_b", name="A_b")
            B_b = abp.tile([P, T], F32, tag="B_b", name="B_b")
            for tch in range(2):
                sl = slice(512 * tch, 512 * (tch + 1))
                for row, dst in ((arow, A_b), (brow, B_b)):
                    bps = psp.tile([P, 512], F32, tag="bc", name="bc")
                    nc.tensor.matmul(bps, ones_1x128, row[:, sl], start=True, stop=True)
                    nc.scalar.copy(out=dst[:, sl], in_=bps)
            return A_b, B_b

        def ln_apply(xtiles, A_b, B_b, gcol, bcol, hpool, tsl=slice(0, T)):
            w = tsl.stop - tsl.start
            htiles = []
            for c in range(NCT):
                t1 = scrp.tile([P, T], F32, tag="lnscr", name="lnscr")
                nc.vector.scalar_tensor_tensor(
                    out=t1[:, 0:w], in0=xtiles[c][:, tsl], scalar=gcol[:, c:c + 1],
                    in1=A_b[:, tsl], op0=OP.mult, op1=OP.mult)
                nc.vector.scalar_tensor_tensor(
                    out=t1[:, 0:w], in0=B_b[:, tsl], scalar=gcol[:, c:c + 1],
                    in1=t1[:, 0:w], op0=OP.mult, op1=OP.add)
                ht = hpool.tile([P, w], F32R, tag="h", name="h")
                nc.scalar.activation(out=ht, in_=t1[:, 0:w], func=AF.Identity,
                                     bias=bcol[:, c:c + 1], scale=1.0)
                htiles.append(ht)
            return htiles

        def attn_chunk(kq_of, vaug_tiles, n_s, otiles, h, tch, psp, ppool, causal):
            (kt, ko), (qt, qo) = kq_of(h)
            tsl = slice(512 * tch, 512 * (tch + 1))
            ptiles = []
            sp_tiles = []
            for st in range(n_s):
                sps = psp.tile([P, 512], F32, tag="s", name="s")
                h0 = h - h % 2  # pack the head pair's S matmuls back to back so
                # their disjoint PE row groups (tile_position) run concurrently
                nc.tensor.matmul(sps, kt[ko:ko + D, st * P:(st + 1) * P],
                                 qt[qo:qo + D, tsl], start=True, stop=True,
                                 tile_position=(ko, 0))
                sp_tiles.append(sps)
            for st in range(n_s):
                sps = sp_tiles[st]
                pt = ppool.tile([P, 512], F32R, tag="p", name="p")
                j = st - 4 * tch
                if causal and j >= 0:
                    z = P * j
                    if z:
                        nc.vector.tensor_copy(out=pt[:, 0:z], in_=zeros384[:, 0:z])
                    nc.scalar.activation(out=pt[:, z:512], in_=sps[:, z:512],
                                         func=AF.Exp, scale=0.125)
                    nc.vector.tensor_tensor(out=pt[:, z:z + P], in0=pt[:, z:z + P],
                                            in1=master[:, 384:512], op=OP.mult)
                else:
                    nc.scalar.activation(out=pt, in_=sps, func=AF.Exp, scale=0.125)
                ptiles.append(pt)
            ops = psp.tile([65, 512], F32, tag="o", name="o")
            for st in range(n_s):
                nc.tensor.matmul(ops, vaug_tiles[st][:, 65 * h:65 * h + 65],
                                 ptiles[st], start=(st == 0), stop=(st == n_s - 1))
            rden = rbp.tile([1, 512], F32R, tag="rden", name="rden")
            nc.vector.reciprocal(out=rden, in_=ops[64:65, :])
            bps = psp.tile([64, 512], F32, tag="b", name="b")
            nc.tensor.matmul(bps, ones_1x128[:, 0:64], rden, start=True, stop=True)
            rb = rbp.tile([64, 512], F32, tag="rb", name="rb")
            nc.scalar.copy(out=rb, in_=bps)
            ot = otiles[h // 2]
            po = (h % 2) * D
            nc.vector.tensor_tensor(out=ot[po:po + D, tsl], in0=ops[0:64, :], in1=rb,
                                    op=OP.mult)

        # ================= P0: load & transpose x, LN1, qkv ==================
        xT_cm, xTp = openp(name="xT", bufs=NCT, side="right")
        h1_cm, hp = openp(name="h1", bufs=NCT)

        tok_cm, tokp = openp(name="tok0", bufs=2)
        tp_cm, tpp = openp(name="psT0", bufs=2, space="PSUM")
        xT = [xTp.tile([P, T], F32R, tag="xT", name="xT") for _ in range(NCT)]
        for tt in range(NTT):
            tok = tokp.tile([P, C], F32, tag="tok", name="tok")
            nc.sync.dma_start(out=tok, in_=dr["x"].ap()[tt * P:(tt + 1) * P, :])
            for c in range(NCT):
                tps = tpp.tile([P, P], F32, tag="tp", name="tp")
                nc.tensor.transpose(tps, tok[:, c * P:(c + 1) * P], ident)
                nc.vector.tensor_copy(out=xT[c][:, tt * P:(tt + 1) * P], in_=tps)
        tp_cm.__exit__(None, None, None)
        tok_cm.__exit__(None, None, None)

        ln_cm, lnp = openp(name="psLN0", bufs=2, space="PSUM")
        A_b, B_b = ln_stats(xT, lnp)
        ln_cm.__exit__(None, None, None)
        h_t = ln_apply(xT, A_b, B_b, g1, b1, hp)
        for c in range(NCT):
            nc.sync.dma_start(out=xT_d[c], in_=xT[c])
        xT_cm.__exit__(None, None, None)

        # v first (x-stationary), then q,k (W-stationary)
        vap_cm, vap = openp(name="vaug", bufs=NTT, side="right")
        vaug = [vap.tile([P, 16 * 65], F32R, tag="va", name="va") for _ in range(NTT)]

        wv_cm, wv = openp(name="wv", bufs=2)
        accv_cm, accv = openp(name="psACv", bufs=3, space="PSUM")
        brow_v = rowp.tile([1, C], F32, tag="row", name="braw")
        nc.sync.dma_start(out=brow_v,
                          in_=dr["b_attn"].ap()[2 * C:3 * C].rearrange("(a c) -> a c", a=1))
        bvb1 = bcast_row(brow_v, wv, accv, "bvb")
        for cc in range(4):   # v output chunks of 256 cols (4 heads each)
            stage = wv.tile([P, NCT, 256], F32, tag="vws", name="vws")
            nc.sync.dma_start(
                out=stage,
                in_=W2d("W_attn")[:, 2 * C + 256 * cc: 2 * C + 256 * (cc + 1)]
                .rearrange("(c p) f -> p c f", p=P))
            wr = wv.tile([P, NCT, 256], F32R, tag="vwr", name="vwr")
            nc.gpsimd.tensor_copy(out=wr, in_=stage)
            for tt in range(NTT):
                vps = accv.tile([P, 256], F32, tag="acc", name="acc")
                for c in range(NCT):
                    nc.tensor.matmul(vps, h_t[c][:, tt * P:(tt + 1) * P], wr[:, c, :],
                                     start=(c == 0), stop=(c == NCT - 1))
                dst = vaug[tt].rearrange("p (h x) -> p h x", x=65)[:, 4 * cc:4 * (cc + 1), 0:64]
                nc.vector.tensor_tensor(
                    out=dst, in0=vps.rearrange("p (h x) -> p h x", x=64),
                    in1=bvb1[:, 256 * cc:256 * (cc + 1)].rearrange("p (h x) -> p h x", x=64),
                    op=OP.add)
        for tt in range(NTT):
            nc.vector.tensor_copy(
                out=vaug[tt].rearrange("p (h x) -> p h x", x=65)[:, :, 64:65],
                in_=ones_col.rearrange("p (h x) -> p h x", x=1))
        accv_cm.__exit__(None, None, None)
        wv_cm.__exit__(None, None, None)

        qk_cm, qkp = openp(name="qk", bufs=16, side="right")
        w1_cm, w1 = openp(name="w1", bufs=2)
        acc_cm, accp = openp(name="psAC1", bufs=3, space="PSUM")
        qk_t = []
        for f in range(16):
            wsl = load_wslab(W2d("W_attn"), f, w1)
            qt = qkp.tile([P, T], F32R, tag="qk", name="qk")
            for tch in range(2):
                aps = accp.tile([P, 512], F32, tag="acc", name="acc")
                for c in range(NCT):
                    nc.tensor.matmul(aps, wsl[:, c, :], h_t[c][:, 512 * tch:512 * (tch + 1)],
                                     start=(c == 0), stop=(c == NCT - 1))
                nc.scalar.activation(out=qt[:, 512 * tch:512 * (tch + 1)], in_=aps,
                                     func=AF.Identity, bias=bqk[:, f:f + 1], scale=1.0)
            qk_t.append(qt)
        acc_cm.__exit__(None, None, None)
        w1_cm.__exit__(None, None, None)
        h1_cm.__exit__(None, None, None)

        # ================= P2: self attention =================
        o_cm, opool = openp(name="o1", bufs=NCT)
        pp_cm, pp = openp(name="pp1", bufs=8)
        psS_cm, psS = openp(name="psS1", bufs=2, space="PSUM")

        otiles = [opool.tile([P, T], F32R, tag="ot", name="ot") for _ in range(NCT)]

        def kq_self(h):
            return (qk_t[8 + h // 2], (h % 2) * D), (qk_t[h // 2], (h % 2) * D)

        for tch in range(2):
            for h in range(H):
                attn_chunk(kq_self, vaug, 4 * (tch + 1), otiles, h, tch, psS, pp,
                           causal=True)

        psS_cm.__exit__(None, None, None)
        pp_cm.__exit__(None, None, None)
        qk_cm.__exit__(None, None, None)
        vap_cm.__exit__(None, None, None)

        # ================= P3: aproj + residual (x1 = x + sa) =================
        res_cm, residp = openp(name="resid", bufs=NCT, side="right")
        resid = [residp.tile([P, T], F32R, tag="res", name="res") for _ in range(NCT)]

        xo_cm, xop = openp(name="xold", bufs=2)
        w2_cm, w2 = openp(name="w2", bufs=2)
        acc_cm, accp = openp(name="psAC3", bufs=3, space="PSUM")
        for co in range(NCT):
            wsl = load_wslab(W2d("W_aproj"), co, w2)
            xold = xop.tile([P, T], F32R, tag="xold", name="xold")
            nc.sync.dma_start(out=xold, in_=xT_d[co])
            for tch in range(2):
                sl = slice(512 * tch, 512 * (tch + 1))
                aps = accp.tile([P, 512], F32, tag="acc", name="acc")
                for c in range(NCT):
                    nc.tensor.matmul(aps, wsl[:, c, :], otiles[c][:, sl],
                                     start=(c == 0), stop=(c == NCT - 1))
                nc.vector.scalar_tensor_tensor(
                    out=resid[co][:, sl], in0=aps, scalar=bap_c[:, co:co + 1],
                    in1=xold[:, sl], op0=OP.add, op1=OP.add)
        acc_cm.__exit__(None, None, None)
        w2_cm.__exit__(None, None, None)
        xo_cm.__exit__(None, None, None)
        o_cm.__exit__(None, None, None)

        # ================= P4: cross attention =================
        ln_cm, lnp = openp(name="psLN1", bufs=2, space="PSUM")
        A_b, B_b = ln_stats(resid, lnp)
        ln_cm.__exit__(None, None, None)

        # right-stack: k2, v2, then q2 (live until end of cross attention)
        k2_cm, k2p = openp(name="k2", bufs=NCT, side="right")
        v2_cm, v2p = openp(name="v2", bufs=2, side="right")

        w3_cm, w3 = openp(name="w3", bufs=2)

        img_cm, imgp = openp(name="img", bufs=NCT)
        tok_cm, tokp = openp(name="tok4", bufs=2)
        tp_cm, tpp = openp(name="psT4", bufs=2, space="PSUM")
        imgT = [imgp.tile([P, TI], F32R, tag="imgT", name="imgT") for _ in range(NCT)]
        for tt in range(TI // P):
            tok = tokp.tile([P, C], F32, tag="tok", name="tok")
            nc.sync.dma_start(out=tok, in_=dr["x_img_feats"].ap()[tt * P:(tt + 1) * P, :])
            for c in range(NCT):
                tps = tpp.tile([P, P], F32, tag="tp", name="tp")
                nc.tensor.transpose(tps, tok[:, c * P:(c + 1) * P], ident)
                nc.vector.tensor_copy(out=imgT[c][:, tt * P:(tt + 1) * P], in_=tps)
        tp_cm.__exit__(None, None, None)
        tok_cm.__exit__(None, None, None)

        acc_cm, accp = openp(name="psAC4", bufs=2, space="PSUM")
        k2_t = []
        for f in range(NCT):
            wsl = load_wslab(W2d("Wk"), f, w3, eng=(nc.vector if f % 2 else nc.gpsimd))
            kt = k2p.tile([P, TI], F32R, tag="k2", name="k2")
            aps = accp.tile([P, 512], F32, tag="acc", name="acc")
            for c in range(NCT):
                nc.tensor.matmul(aps[:, 0:TI], wsl[:, c, :], imgT[c],
                                 start=(c == 0), stop=(c == NCT - 1))
            nc.scalar.activation(out=kt, in_=aps[:, 0:TI], func=AF.Identity,
                                 bias=bk_c[:, f:f + 1], scale=1.0)
            k2_t.append(kt)

        brow_v2 = rowp.tile([1, C], F32, tag="row", name="braw2")
        nc.sync.dma_start(out=brow_v2, in_=dr["bv"].ap().rearrange("(a c) -> a c", a=1))
        wv2_cm, wv2 = openp(name="wv2", bufs=2)
        bvb2 = bcast_row(brow_v2, wv2, accp, "bvb2")

        v2aug = [v2p.tile([P, 16 * 65], F32R, tag="va2", name="va2")
                 for _ in range(TI // P)]
        for cc in range(4):
            stage = wv2.tile([P, NCT, 256], F32, tag="vws", name="vws")
            nc.sync.dma_start(
                out=stage,
                in_=W2d("Wv")[:, 256 * cc: 256 * (cc + 1)].rearrange("(c p) f -> p c f", p=P))
            wr = wv2.tile([P, NCT, 256], F32R, tag="vwr", name="vwr")
            nc.gpsimd.tensor_copy(out=wr, in_=stage)
            for st in range(TI // P):
                vps = accp.tile([P, 256], F32, tag="acc2", name="acc2")
                for c in range(NCT):
                    nc.tensor.matmul(vps, imgT[c][:, st * P:(st + 1) * P], wr[:, c, :],
                                     start=(c == 0), stop=(c == NCT - 1))
                dst = v2aug[st].rearrange("p (h x) -> p h x", x=65)[:, 4 * cc:4 * (cc + 1), 0:64]
                nc.vector.tensor_tensor(
                    out=dst, in0=vps.rearrange("p (h x) -> p h x", x=64),
                    in1=bvb2[:, 256 * cc:256 * (cc + 1)].rearrange("p (h x) -> p h x", x=64),
                    op=OP.add)
        for st in range(TI // P):
            nc.vector.tensor_copy(
                out=v2aug[st].rearrange("p (h x) -> p h x", x=65)[:, :, 64:65],
                in_=ones_col.rearrange("p (h x) -> p h x", x=1))
        wv2_cm.__exit__(None, None, None)
        img_cm.__exit__(None, None, None)

        q2_cm, q2p = openp(name="q2", bufs=NCT, side="right")
        hp_cm, hp = openp(name="h2", bufs=NCT)
        hb_t = ln_apply(resid, A_b, B_b, g1, b1, hp)
        q2_t = []
        for f in range(NCT):
            wsl = load_wslab(W2d("Wq"), f, w3)
            qt = q2p.tile([P, T], F32R, tag="q2", name="q2")
            for tch in range(2):
                aps = accp.tile([P, 512], F32, tag="acc", name="acc")
                for c in range(NCT):
                    nc.tensor.matmul(aps, wsl[:, c, :], hb_t[c][:, 512 * tch:512 * (tch + 1)],
                                     start=(c == 0), stop=(c == NCT - 1))
                nc.scalar.activation(out=qt[:, 512 * tch:512 * (tch + 1)], in_=aps,
                                     func=AF.Identity, bias=bq_c[:, f:f + 1], scale=1.0)
            q2_t.append(qt)
        hp_cm.__exit__(None, None, None)
        acc_cm.__exit__(None, None, None)
        w3_cm.__exit__(None, None, None)

        o_cm, opool = openp(name="o2", bufs=NCT)
        pp_cm, pp = openp(name="pp2", bufs=6)
        psS_cm, psS = openp(name="psS2", bufs=2, space="PSUM")

        o2tiles = [opool.tile([P, T], F32R, tag="ot", name="ot") for _ in range(NCT)]

        def kq_cross(h):
            return (k2_t[h // 2], (h % 2) * D), (q2_t[h // 2], (h % 2) * D)

        for tch in range(2):
            for h in range(H):
                attn_chunk(kq_cross, v2aug, TI // P, o2tiles, h, tch, psS, pp,
                           causal=False)

        psS_cm.__exit__(None, None, None)
        pp_cm.__exit__(None, None, None)
        q2_cm.__exit__(None, None, None)
        v2_cm.__exit__(None, None, None)
        k2_cm.__exit__(None, None, None)

        # ================= P5: cproj + residual (x2, in place) =================
        w4_cm, w4 = openp(name="w4", bufs=2)
        acc_cm, accp = openp(name="psAC5", bufs=3, space="PSUM")
        for co in range(NCT):
            wsl = load_wslab(W2d("Wcproj"), co, w4)
            for tch in range(2):
                sl = slice(512 * tch, 512 * (tch + 1))
                aps = accp.tile([P, 512], F32, tag="acc", name="acc")
                for c in range(NCT):
                    nc.tensor.matmul(aps, wsl[:, c, :], o2tiles[c][:, sl],
                                     start=(c == 0), stop=(c == NCT - 1))
                nc.vector.scalar_tensor_tensor(
                    out=resid[co][:, sl], in0=aps, scalar=bcp_c[:, co:co + 1],
                    in1=resid[co][:, sl], op0=OP.add, op1=OP.add)
        acc_cm.__exit__(None, None, None)
        w4_cm.__exit__(None, None, None)
        o_cm.__exit__(None, None, None)

        # ================= P6: MLP =================
        ln_cm, lnp = openp(name="psLN2", bufs=2, space="PSUM")
        A_b, B_b = ln_stats(resid, lnp)
        ln_cm.__exit__(None, None, None)

        up_cm, up = openp(name="u", bufs=16, side="right")

        for tch in range(2):
            tsl = slice(512 * tch, 512 * (tch + 1))
            hp_cm, hp = openp(name=f"h3{tch}", bufs=NCT)
            h2_t = ln_apply(resid, A_b, B_b, g2, b2, hp, tsl=tsl)
            utiles = [up.tile([P, 2, 512], F32R, tag="u", name="u") for _ in range(16)]
            w5_cm, w5 = openp(name=f"w5{tch}", bufs=3)
            acc_cm, accp = openp(name=f"psU{tch}", bufs=2, space="PSUM")
            for ff in range(NFT):
                wsl = load_wslab(W2d("W_fc"), ff, w5,
                                 eng=(nc.vector if ff % 2 else nc.gpsimd))
                ups = accp.tile([P, 512], F32, tag="acc", name="acc")
                for c in range(NCT):
                    nc.tensor.matmul(ups, wsl[:, c, :], h2_t[c],
                                     start=(c == 0), stop=(c == NCT - 1))
                nc.scalar.activation(out=utiles[ff // 2][:, ff % 2, :], in_=ups,
                                     func=AF.Gelu_apprx_tanh,
                                     bias=bfc_c[:, ff:ff + 1], scale=1.0)
            acc_cm.__exit__(None, None, None)
            w5_cm.__exit__(None, None, None)
            hp_cm.__exit__(None, None, None)

            w6_cm, w6 = openp(name=f"w6{tch}", bufs=4)
            psM_cm, psM = openp(name=f"psM{tch}", bufs=8, space="PSUM")
            mps = [psM.tile([P, 512], F32, tag="m", name="m") for _ in range(NCT)]
            for ff in range(NFT):
                stage = w6.tile([P, C], F32, tag="mps", name="mps")
                nc.sync.dma_start(out=stage, in_=W2d("W_mproj")[ff * P:(ff + 1) * P, :])
                wr = w6.tile([P, C], F32R, tag="mpr", name="mpr")
                (nc.vector if ff % 2 else nc.gpsimd).tensor_copy(out=wr, in_=stage)
                for co in range(NCT):
                    nc.tensor.matmul(mps[co], wr[:, co * P:(co + 1) * P],
                                     utiles[ff // 2][:, ff % 2, :],
                                     start=(ff == 0), stop=(ff == NFT - 1))
            for co in range(NCT):
                nc.vector.scalar_tensor_tensor(
                    out=resid[co][:, tsl], in0=mps[co], scalar=bmp_c[:, co:co + 1],
                    in1=resid[co][:, tsl], op0=OP.add, op1=OP.add)
            psM_cm.__exit__(None, None, None)
            w6_cm.__exit__(None, None, None)

        up_cm.__exit__(None, None, None)

        # ================= P7: transpose back & store =================
        tok_cm, tokp = openp(name="tok7", bufs=2)
        tp_cm, tpp = openp(name="psT7", bufs=4, space="PSUM")
        for tt in range(NTT):
            otok = tokp.tile([P, C], F32, tag="tok", name="tok")
            for c in range(NCT):
                tps = tpp.tile([P, P], F32R, tag="tpr", name="tpr")
                nc.tensor.transpose(tps, resid[c][:, tt * P:(tt + 1) * P], identR)
                nc.vector.tensor_copy(out=otok[:, c * P:(c + 1) * P], in_=tps)
            nc.sync.dma_start(out=out_d.ap()[tt * P:(tt + 1) * P, :], in_=otok)
        tp_cm.__exit__(None, None, None)
        tok_cm.__exit__(None, None, None)
        res_cm.__exit__(None, None, None)

        for cm in reversed(kw_cms):
            cm.__exit__(None, None, None)

    nc.compile()
    return nc


def kernel(**inputs):
    from concourse.bass_utils import run_bass_kernel_spmd

    if "nc" not in _CACHED:
        _CACHED["nc"] = _build()
    nc = _CACHED["nc"]

    np_inputs = {k: np.asarray(v, dtype=np.float32) for k, v in inputs.items()}
    in_maps = []
    for b in range(B):
        m = dict(np_inputs)
        m["x"] = np.ascontiguousarray(np_inputs["x"][b])
        m["x_img_feats"] = np.ascontiguousarray(np_inputs["x_img_feats"][b])
        in_maps.append(m)
    res = run_bass_kernel_spmd(nc, in_maps, core_ids=list(range(B)))
    out = np.stack([res.results[b]["out"] for b in range(B)], axis=0)
    return out.astype(np.float32)



# revision 31
# speedup vs baseline: 1.3175x; 1.3175x over previous
"""Trainium2 Bass kernel for a dense transformer block (self-attn + cross-attn + MLP).

Sharding: data-parallel over batch, one batch element per NeuronCore (B=8, 8 cores),
no collectives. All activations are kept feature-major ([C, T]) on chip so every
projection matmul consumes weight tiles directly from DRAM.

Perf structure:
- bf16 weights (host-cast) stream straight into matmul-ready SBUF tiles as
  256-column pairs (512B DRAM rows per descriptor); no on-chip cast pass.
- Matmul moving operands (LN outputs, q/k, attention probabilities, gelu
  outputs) are bf16: full PE rate at any moving width. Residual stays f32r,
  carried in-place in the xT tiles.
- Softmax: S^T layout; denominator from a ones column in the augmented V; the
  reciprocal row is partition-broadcast on gpsimd (not PE+Act).
- Causal attention trims S/AV matmuls and exp to the unmasked column span.
- Emission interleaves independent PE work into the Act-bound attention
  windows: cross-attn img transpose + K/V projections run during self-attn;
  a_proj / q-proj for token chunk 0 run during chunk 1's attention; cproj
  chunk 0 runs during cross-attn chunk 1; transpose-out of chunk 0 runs
  during the MLP's chunk 1.
- Biases load as [nf,128] row tiles (contiguous DMA) and are PE-transposed to
  per-partition columns, avoiding 4-byte-descriptor DMA storms.
"""

import sys
import numpy as np

sys.path.insert(0, "/opt/trn_rl_repo")

B, T, C = 8, 1024, 1024
H = 16
D = C // H          # 64
TI = 256
FF = 4 * C          # 4096
EPS = 1e-5
NCT = C // 128      # 8 c tiles
NTT = T // 128      # 8 t tiles
NFT = FF // 128     # 32 ff tiles
P = 128

_CACHED = {}

WEIGHT_NAMES = ("W_attn", "W_aproj", "Wq", "Wk", "Wv", "Wcproj", "W_fc", "W_mproj")


def _build():
    import concourse.tile as tile
    from concourse import bacc, mybir
    from concourse.masks import make_identity

    F32, F32R, BF16 = mybir.dt.float32, mybir.dt.float32r, mybir.dt.bfloat16
    AF = mybir.ActivationFunctionType
    OP = mybir.AluOpType

    nc = bacc.Bacc("TRN2", target_bir_lowering=False, debug=False, num_devices=8)

    dr = {}
    dr["x"] = nc.dram_tensor("x", [T, C], F32, kind="ExternalInput")
    dr["x_img_feats"] = nc.dram_tensor("x_img_feats", [TI, C], F32, kind="ExternalInput")
    for nm, shp in [
        ("ln1_g", [C]), ("ln1_b", [C]), ("ln2_g", [C]), ("ln2_b", [C]),
        ("b_attn", [3 * C]), ("b_aproj", [C]),
        ("bq", [C]), ("bk", [C]), ("bv", [C]), ("bcproj", [C]),
        ("b_fc", [FF]), ("b_mproj", [C]),
    ]:
        dr[nm] = nc.dram_tensor(nm, shp, F32, kind="ExternalInput")
    for nm, shp in [
        ("W_attn", [C, 3 * C]), ("W_aproj", [C, C]),
        ("Wq", [C, C]), ("Wk", [C, C]), ("Wv", [C, C]), ("Wcproj", [C, C]),
        ("W_fc", [C, FF]), ("W_mproj", [FF, C]),
    ]:
        dr[nm] = nc.dram_tensor(nm, shp, BF16, kind="ExternalInput")
    out_d = nc.dram_tensor("out", [T, C], F32, kind="ExternalOutput")

    def W2d(name):
        return dr[name].ap()

    with tile.TileContext(nc) as tc, nc.allow_low_precision(
        reason="bf16 weights/activations are within the 2e-2 tolerance"
    ):
        kw_cms = []

        def openp(**kw):
            cm = tc.tile_pool(**kw)
            return cm, cm.__enter__()

        def openkw(**kw):
            cm, p = openp(**kw)
            kw_cms.append(cm)
            return p

        # ---------------- kernel-wide pools (left-stack base) ----------------
        constp = openkw(name="const", bufs=1)
        fsrp = openkw(name="fsr", bufs=2)       # f32r [128,512] squares
        abp = openkw(name="ab", bufs=1)         # A_b/B_b [128,1024]
        rowp = openkw(name="rows", bufs=4)      # one [1,1024] "row" tag
        rbp = openkw(name="rb", bufs=2)         # [64,512] + [1,512] rden

        # ---------------- constants ----------------
        identB = constp.tile([P, P], BF16)
        ones_col = constp.tile([P, 16], BF16)
        nc.vector.memset(ones_col, 1.0)
        ones_f = constp.tile([P, 1], F32)
        nc.vector.memset(ones_f, 1.0)
        ones128R = constp.tile([P, 1], F32R)
        nc.vector.tensor_copy(out=ones128R, in_=ones_f)
        eps_t = constp.tile([1, 1], F32)
        nc.vector.memset(eps_t, EPS)

        # diagonal causal mask [128, 128]: keep (1.0) iff col >= row.
        master = constp.tile([P, P], BF16)
        nc.gpsimd.memset(master, 1.0)
        nc.gpsimd.affine_select(
            out=master, in_=master, compare_op=OP.is_ge, fill=0.0,
            base=0, pattern=[[1, P]], channel_multiplier=-1)

        # bias/gain columns: contiguous [nf,128] row DMA + one PE transpose
        psB_cm, psB = openp(name="psB", bufs=2, space="PSUM")
        stg_cm, stgp = openp(name="bstage", bufs=3)
        idf = constp.tile([P, P], F32)
        make_identity(nc, idf)
        nc.vector.tensor_copy(out=identB, in_=idf)
        identR = constp.tile([P, P], F32R)
        nc.gpsimd.tensor_copy(out=identR, in_=idf)

        def load_cols(name, nf, lo=0):
            row = stgp.tile([1, nf * P], F32, tag="bstg", name=name + "_s")
            nc.sync.dma_start(
                out=row,
                in_=dr[name].ap()[lo:lo + nf * P].rearrange("(a c) -> a c", a=1))
            tps = psB.tile([P, nf], F32, tag="bt", name="bt")
            for i in range(nf):
                nc.tensor.transpose(tps[:, i:i + 1], row[0:1, i * P:(i + 1) * P],
                                    idf[0:1, 0:1])
            t = constp.tile([P, nf], F32, name=name + "_c")
            nc.vector.tensor_copy(out=t, in_=tps)
            return t

        bqk = load_cols("b_attn", 16)
        bq_c = load_cols("bq", NCT)
        bk_c = load_cols("bk", NCT)
        bap_c = load_cols("b_aproj", NCT)
        bcp_c = load_cols("bcproj", NCT)
        bmp_c = load_cols("b_mproj", NCT)
        bfc_c = load_cols("b_fc", NFT)
        stg_cm.__exit__(None, None, None)
        psB_cm.__exit__(None, None, None)

        # ---------------- helpers ----------------
        def bcast_row_bf16(row_f32, dest_pool, tag):
            rowb = rowp.tile([1, C], BF16, tag="row", name="rowb")
            nc.vector.tensor_copy(out=rowb, in_=row_f32)
            dest = dest_pool.tile([P, C], BF16, tag=tag, name=tag)
            nc.gpsimd.partition_broadcast(dest, rowb)
            return dest

        def load_wpair(wap, co2, wpool):
            """W[:, co2*256:(co2+1)*256] bf16 -> [128, K/128, 256] slab."""
            nk = wap.shape[0] // P
            wr = wpool.tile([P, nk, 256], BF16, tag="ws", name="ws")
            nc.sync.dma_start(
                out=wr,
                in_=wap[:, co2 * 256:(co2 + 1) * 256]
                .rearrange("(c p) f -> p c f", p=P))
            return wr

        def ln_stats_tch(xtiles, psp, tch, A_b, B_b):
            sl = slice(512 * tch, 512 * (tch + 1))
            sp = psp.tile([1, 512], F32, tag="lnsum", name="lnsum")
            qp = psp.tile([1, 512], F32, tag="lnsq", name="lnsq")
            for c in range(NCT):
                xs = xtiles[c][:, sl]
                nc.tensor.matmul(sp, ones128R, xs, start=(c == 0), stop=(c == NCT - 1))
                sq = fsrp.tile([P, 512], F32R, tag="sq", name="sq")
                if c % 2:
                    nc.scalar.activation(out=sq, in_=xs, func=AF.Square, scale=1.0)
                else:
                    nc.vector.tensor_tensor(out=sq, in0=xs, in1=xs, op=OP.mult)
                nc.tensor.matmul(qp, ones128R, sq, start=(c == 0), stop=(c == NCT - 1))
            mu = rowp.tile([1, 512], F32, tag="row", name="mu")
            msq = rowp.tile([1, 512], F32, tag="row", name="msq")
            nc.vector.tensor_scalar_mul(out=mu, in0=sp, scalar1=1.0 / C)
            nc.vector.tensor_scalar_mul(out=msq, in0=qp, scalar1=1.0 / C)
            musq = rowp.tile([1, 512], F32, tag="row", name="musq")
            nc.vector.tensor_tensor(out=musq, in0=mu, in1=mu, op=OP.mult)
            nc.vector.tensor_tensor(out=msq, in0=msq, in1=musq, op=OP.subtract)
            nc.scalar.activation(out=musq, in_=msq, func=AF.Sqrt, bias=eps_t, scale=1.0)
            arow = rowp.tile([1, 512], BF16, tag="row", name="arow")
            nc.vector.reciprocal(out=arow, in_=musq)
            brow = rowp.tile([1, 512], BF16, tag="row", name="brow")
            nc.vector.scalar_tensor_tensor(out=brow, in0=mu, scalar=-1.0, in1=arow,
                                           op0=OP.mult, op1=OP.mult)
            nc.gpsimd.partition_broadcast(A_b[:, sl], arow)
            nc.gpsimd.partition_broadcast(B_b[:, sl], brow)

        def ln_ab():
            A_b = abp.tile([P, T], BF16, tag="A_b", name="A_b")
            B_b = abp.tile([P, T], BF16, tag="B_b", name="B_b")
            return A_b, B_b

        def ln_apply(xtiles, A_b, B_b, hpool, tsl=slice(0, T)):
            # ht = x*A_b + B_b (LN gain/bias are folded into the consuming
            # weights host-side). Two plain tensor_tensor ops, in place in the
            # bf16 h tile, alternating gpsimd/DVE.
            w = tsl.stop - tsl.start
            htiles = []
            for c in range(NCT):
                e2 = nc.gpsimd if c % 2 == 0 else nc.vector
                ht = hpool.tile([P, w], BF16, tag="h", name="h")
                nc.vector.tensor_tensor(out=ht, in0=xtiles[c][:, tsl], in1=A_b[:, tsl],
                                        op=OP.mult)
                e2.tensor_tensor(out=ht, in0=ht, in1=B_b[:, tsl], op=OP.add)
                htiles.append(ht)
            return htiles

        def attn_chunk(kq_of, vaug_tiles, n_s, otiles, h, tch, psp, ppool, causal):
            (kt, ko), (qt, qo) = kq_of(h)
            base = 512 * tch
            sp_tiles, offs = [], []
            for st in range(n_s):
                off = max(0, P * st - base) if causal else 0
                sps = psp.tile([P, 512], F32, tag="s", name="s")
                nc.tensor.matmul(sps[:, off:512], kt[ko:ko + D, st * P:(st + 1) * P],
                                 qt[qo:qo + D, base + off:base + 512],
                                 start=True, stop=True, tile_position=(ko, 0))
                sp_tiles.append(sps)
                offs.append(off)
            ptiles = []
            for st in range(n_s):
                off = offs[st]
                pt = ppool.tile([P, 512], BF16, tag="p", name="p")
                nc.scalar.activation(out=pt[:, off:512], in_=sp_tiles[st][:, off:512],
                                     func=AF.Exp, scale=0.125)
                if causal and P * st - base >= 0:
                    nc.vector.tensor_tensor(out=pt[:, off:off + P], in0=pt[:, off:off + P],
                                            in1=master, op=OP.mult)
                ptiles.append(pt)
            ops = psp.tile([65, 512], F32, tag="o", name="o")
            for st in range(n_s):
                off = offs[st]
                nc.tensor.matmul(ops[:, off:512], vaug_tiles[st][:, 65 * h:65 * h + 65],
                                 ptiles[st][:, off:512], start=(st == 0),
                                 stop=(st == n_s - 1), skip_group_check=True)
            rden = rbp.tile([1, 512], BF16, tag="rden", name="rden")
            nc.vector.reciprocal(out=rden, in_=ops[64:65, :])
            rb = rbp.tile([64, 512], BF16, tag="rb", name="rb")
            nc.gpsimd.partition_broadcast(rb, rden)
            ot = otiles[h // 2]
            po = (h % 2) * D
            nc.vector.tensor_tensor(
                out=ot[po:po + D, base:base + 512],
                in0=ops[0:64, :], in1=rb, op=OP.mult)

        # ===== psLN psum pool: LN statistics banks, lives until the MLP =====
        psLN_cm, psLN = openp(name="psLN", bufs=1, space="PSUM")

        # ========== right stack: xT doubles as the residual stream ==========
        xT_cm, xTp = openp(name="xT", bufs=NCT, side="right")
        xT = [xTp.tile([P, T], F32R, tag="xT", name="xT") for _ in range(NCT)]

        # imgT lives (left) from P0 until cross attention ends
        img_cm, imgp = openp(name="img", bufs=NCT)
        imgT = [imgp.tile([P, TI], BF16, tag="imgT", name="imgT") for _ in range(NCT)]

        # ================= P0: load & transpose x and img =================
        h1_cm, hp = openp(name="h1", bufs=2 * NCT)
        tok_cm, tokp = openp(name="tok0", bufs=3)
        tp_cm, tpp = openp(name="psT0", bufs=6, space="PSUM")
        cpeng = [nc.vector, nc.scalar, nc.gpsimd]

        def transpose_tok(src_ap, dst_tiles, tt, dst_off):
            tok = tokp.tile([P, C], F32, tag="tok", name="tok")
            nc.sync.dma_start(out=tok, in_=src_ap[tt * P:(tt + 1) * P, :])
            for c in range(NCT):
                tps = tpp.tile([P, P], F32, tag="tp", name="tp")
                nc.tensor.transpose(tps, tok[:, c * P:(c + 1) * P], idf)
                sl = slice(dst_off + tt * P, dst_off + (tt + 1) * P)
                if (tt * NCT + c) % 2:
                    nc.scalar.copy(out=dst_tiles[c][:, sl], in_=tps)
                else:
                    nc.vector.tensor_copy(out=dst_tiles[c][:, sl], in_=tps)

        for tt in range(NTT):
            transpose_tok(dr["x"].ap(), xT, tt, 0)
        for tt in range(TI // P):
            transpose_tok(dr["x_img_feats"].ap(), imgT, tt, 0)

        A_b, B_b = ln_ab()
        h_t = [[None] * NCT, [None] * NCT]
        for tch in range(2):
            ln_stats_tch(xT, psLN, tch, A_b, B_b)
            h_t[tch][:] = ln_apply(xT, A_b, B_b, hp,
                                   tsl=slice(512 * tch, 512 * (tch + 1)))
        tp_cm.__exit__(None, None, None)
        tok_cm.__exit__(None, None, None)

        # ====== merged qkv + self-attention (+ hoisted cross-attn prep) ======
        vap_cm, vap = openp(name="vaug", bufs=NTT, side="right")
        vaug = [vap.tile([P, 16 * 65], BF16, tag="va", name="va") for _ in range(NTT)]
        qk_cm, qkp = openp(name="qk", bufs=16, side="right")
        qk_t = [qkp.tile([P, T], BF16, tag="qk", name="qk") for _ in range(16)]

        k2_cm, k2p = openp(name="k2", bufs=NCT)
        v2_cm, v2p = openp(name="v2", bufs=2)
        hb_cm, hbp = openp(name="hb", bufs=NCT)
        q2_cm, q2p = openp(name="q2", bufs=NCT)
        w23_cm, w23 = openp(name="w23", bufs=2)
        psAC_cm, accp = openp(name="psAC", bufs=2, space="PSUM")

        o_cm, opool = openp(name="o1", bufs=NCT)
        pp_cm, pp = openp(name="pp1", bufs=5)
        psS_cm, psS = openp(name="psS1", bufs=2, space="PSUM")

        otiles = [opool.tile([P, T], BF16, tag="ot", name="ot") for _ in range(NCT)]
        k2_t = [k2p.tile([P, TI], BF16, tag="k2", name="k2") for _ in range(NCT)]
        v2aug = [v2p.tile([P, 16 * 65], BF16, tag="va2", name="va2")
                 for _ in range(TI // P)]
        q2_t = [q2p.tile([P, T], BF16, tag="q2", name="q2") for _ in range(NCT)]

        def kq_self(h):
            return (qk_t[8 + h // 2], (h % 2) * D), (qk_t[h // 2], (h % 2) * D)

        def kq_cross(h):
            return (k2_t[h // 2], (h % 2) * D), (q2_t[h // 2], (h % 2) * D)

        side = []

        def drain(n=1):
            for _ in range(n):
                if side:
                    side.pop(0)()

        brow_v = rowp.tile([1, C], F32, tag="row", name="braw")
        nc.sync.dma_start(out=brow_v,
                          in_=dr["b_attn"].ap()[2 * C:3 * C].rearrange("(a c) -> a c", a=1))
        bvb1 = bcast_row_bf16(brow_v, w23, "bvb")
        brow_v2 = rowp.tile([1, C], F32, tag="row", name="braw2")
        nc.sync.dma_start(out=brow_v2, in_=dr["bv"].ap().rearrange("(a c) -> a c", a=1))
        bvb2 = bcast_row_bf16(brow_v2, w23, "bvb2")
        for tt in range(NTT):
            nc.gpsimd.tensor_copy(
                out=vaug[tt].rearrange("p (h x) -> p h x", x=65)[:, :, 64:65],
                in_=ones_col.rearrange("p (h x) -> p h x", x=1))

        def v_group(cc, vtch):
            wr = load_wpair(W2d("W_attn"), 8 + cc, w23)
            for tt in range(4 * vtch, 4 * vtch + 4):
                vps = accp.tile([P, 512], F32, tag="acc", name="acc")[:, 0:256]
                for c in range(NCT):
                    nc.tensor.matmul(
                        vps, h_t[tt // 4][c][:, (tt % 4) * P:(tt % 4 + 1) * P],
                        wr[:, c, :], start=(c == 0), stop=(c == NCT - 1))
                dst = vaug[tt].rearrange("p (h x) -> p h x", x=65)[:, 4 * cc:4 * (cc + 1), 0:64]
                nc.vector.tensor_tensor(
                    out=dst, in0=vps.rearrange("p (h x) -> p h x", x=64),
                    in1=bvb1[:, 256 * cc:256 * (cc + 1)].rearrange("p (h x) -> p h x", x=64),
                    op=OP.add)

        def qk_pair(p_idx, dst0, bias0, tch):
            wsl = load_wpair(W2d("W_attn"), p_idx, w23)
            for fh in range(2):
                qt = qk_t[dst0 + fh]
                aps = accp.tile([P, 512], F32, tag="acc", name="acc")
                for c in range(NCT):
                    nc.tensor.matmul(aps, wsl[:, c, 128 * fh:128 * (fh + 1)],
                                     h_t[tch][c],
                                     start=(c == 0), stop=(c == NCT - 1))
                nc.scalar.activation(out=qt[:, 512 * tch:512 * (tch + 1)], in_=aps,
                                     func=AF.Identity,
                                     bias=bqk[:, bias0 + fh:bias0 + fh + 1], scale=1.0)

        def k2_group(f2):
            def go():
                wsl = load_wpair(W2d("Wk"), f2, w23)
                for fh in range(2):
                    f = 2 * f2 + fh
                    aps = accp.tile([P, 512], F32, tag="acc", name="acc")
                    for c in range(NCT):
                        nc.tensor.matmul(aps[:, 0:TI], wsl[:, c, 128 * fh:128 * (fh + 1)],
                                         imgT[c], start=(c == 0), stop=(c == NCT - 1))
                    nc.vector.tensor_scalar(out=k2_t[f], in0=aps[:, 0:TI],
                                            scalar1=bk_c[:, f:f + 1], scalar2=None,
                                            op0=OP.add)
            return go

        def v2_group(cc):
            def go():
                wr = load_wpair(W2d("Wv"), cc, w23)
                for st in range(TI // P):
                    vps = accp.tile([P, 512], F32, tag="acc", name="acc")[:, 0:256]
                    for c in range(NCT):
                        nc.tensor.matmul(vps, imgT[c][:, st * P:(st + 1) * P], wr[:, c, :],
                                         start=(c == 0), stop=(c == NCT - 1))
                    dst = v2aug[st].rearrange("p (h x) -> p h x", x=65)[:, 4 * cc:4 * (cc + 1), 0:64]
                    nc.vector.tensor_tensor(
                        out=dst, in0=vps.rearrange("p (h x) -> p h x", x=64),
                        in1=bvb2[:, 256 * cc:256 * (cc + 1)].rearrange("p (h x) -> p h x", x=64),
                        op=OP.add)
            return go

        def v2_ones():
            for st in range(TI // P):
                nc.gpsimd.tensor_copy(
                    out=v2aug[st].rearrange("p (h x) -> p h x", x=65)[:, :, 64:65],
                    in_=ones_col.rearrange("p (h x) -> p h x", x=1))

        # ---- post-attention pipeline: aproj, ln1b, q2, cross attention ----
        def aproj_co2(co2, tch):
            def go():
                sl = slice(512 * tch, 512 * (tch + 1))
                wsl = load_wpair(W2d("W_aproj"), co2, w23)
                for ch in range(2):
                    co = 2 * co2 + ch
                    aps = accp.tile([P, 512], F32, tag="acc", name="acc")
                    for c in range(NCT):
                        nc.tensor.matmul(aps, wsl[:, c, 128 * ch:128 * (ch + 1)],
                                         otiles[c][:, sl],
                                         start=(c == 0), stop=(c == NCT - 1))
                    nc.vector.scalar_tensor_tensor(
                        out=xT[co][:, sl], in0=aps, scalar=bap_c[:, co:co + 1],
                        in1=xT[co][:, sl], op0=OP.add, op1=OP.add)
            return go

        A_b2, B_b2 = ln_ab()

        def ln1b_tch(tch):
            def go():
                ln_stats_tch(xT, psLN, tch, A_b2, B_b2)
            return go

        hb_t = [[None] * NCT for _ in range(2)]

        def hb_tch(tch):
            def go():
                hb_t[tch][:] = ln_apply(
                    xT, A_b2, B_b2, hbp,
                    tsl=slice(512 * tch, 512 * (tch + 1)))
            return go

        def q2_group(f2, tch):
            def go():
                wsl = load_wpair(W2d("Wq"), f2, w23)
                for fh in range(2):
                    f = 2 * f2 + fh
                    aps = accp.tile([P, 512], F32, tag="acc", name="acc")
                    for c in range(NCT):
                        nc.tensor.matmul(aps, wsl[:, c, 128 * fh:128 * (fh + 1)],
                                         hb_t[tch][c],
                                         start=(c == 0), stop=(c == NCT - 1))
                    nc.scalar.activation(
                        out=q2_t[f][:, 512 * tch:512 * (tch + 1)], in_=aps,
                        func=AF.Identity, bias=bq_c[:, f:f + 1], scale=1.0)
            return go

        side += [k2_group(f2) for f2 in range(4)]
        side += [v2_group(cc) for cc in range(4)]
        side.append(v2_ones)

        for tch in range(2):
            if tch == 1:
                side += [aproj_co2(co2, 0) for co2 in range(4)]
                side += [ln1b_tch(0), hb_tch(0)]
                side += [q2_group(f2, 0) for f2 in range(4)]
            for g in range(4):
                v_group(g, tch)
                qk_pair(g, 2 * g, 2 * g, tch)
                qk_pair(4 + g, 8 + 2 * g, 8 + 2 * g, tch)
                for h in range(4 * g, 4 * g + 4):
                    attn_chunk(kq_self, vaug, 4 * (tch + 1), otiles, h, tch, psS, pp,
                               causal=True)
                    drain(1)
        drain(len(side))
        qk_cm.__exit__(None, None, None)
        vap_cm.__exit__(None, None, None)

        # ---- cross attention (q2 written post-attention) ----
        psS2 = psS
        pp2 = pp
        o2_cm, opool2 = openp(name="o2", bufs=NCT)
        o2tiles = [opool2.tile([P, T], BF16, tag="ot", name="ot") for _ in range(NCT)]

        def cproj_co2(co2, tch):
            def go():
                sl = slice(512 * tch, 512 * (tch + 1))
                wsl = load_wpair(W2d("Wcproj"), co2, w23)
                for ch in range(2):
                    co = 2 * co2 + ch
                    aps = accp.tile([P, 512], F32, tag="acc", name="acc")
                    for c in range(NCT):
                        nc.tensor.matmul(aps, wsl[:, c, 128 * ch:128 * (ch + 1)],
                                         o2tiles[c][:, sl],
                                         start=(c == 0), stop=(c == NCT - 1))
                    nc.vector.scalar_tensor_tensor(
                        out=xT[co][:, sl], in0=aps, scalar=bcp_c[:, co:co + 1],
                        in1=xT[co][:, sl], op0=OP.add, op1=OP.add)
            return go

        side = [aproj_co2(co2, 1) for co2 in range(4)]
        side += [ln1b_tch(1), hb_tch(1)]
        side += [q2_group(f2, 1) for f2 in range(4)]
        for h in range(H):
            attn_chunk(kq_cross, v2aug, TI // P, o2tiles, h, 0, psS2, pp2,
                       causal=False)
            drain(1)
        drain(len(side))
        side = [cproj_co2(co2, 0) for co2 in range(4)]
        for h in range(H):
            attn_chunk(kq_cross, v2aug, TI // P, o2tiles, h, 1, psS2, pp2,
                       causal=False)
            drain(1)
        drain(len(side))
        for co2 in range(4):
            cproj_co2(co2, 1)()

        o2_cm.__exit__(None, None, None)
        psS_cm.__exit__(None, None, None)
        pp_cm.__exit__(None, None, None)
        o_cm.__exit__(None, None, None)
        psAC_cm.__exit__(None, None, None)
        w23_cm.__exit__(None, None, None)
        q2_cm.__exit__(None, None, None)
        hb_cm.__exit__(None, None, None)
        v2_cm.__exit__(None, None, None)
        k2_cm.__exit__(None, None, None)
        h1_cm.__exit__(None, None, None)
        img_cm.__exit__(None, None, None)

        # ================= P6: MLP =================
        A_b3, B_b3 = ln_ab()
        ln_stats_tch(xT, psLN, 0, A_b3, B_b3)
        ln_stats_tch(xT, psLN, 1, A_b3, B_b3)
        psLN_cm.__exit__(None, None, None)

        up_cm, up = openp(name="u", bufs=16, side="right")
        h3_cm, h3p = openp(name="h3", bufs=2 * NCT)
        h2_both = [ln_apply(xT, A_b3, B_b3, h3p,
                            tsl=slice(512 * t, 512 * (t + 1))) for t in range(2)]

        def transpose_out(tch):
            tok_cm, tokp = openp(name=f"tok7{tch}", bufs=2)
            tp_cm, tpp = openp(name=f"psT7{tch}", bufs=4, space="PSUM")
            for tt in range(4 * tch, 4 * (tch + 1)):
                otok = tokp.tile([P, C], F32, tag="tok", name="tok")
                for c in range(NCT):
                    tps = tpp.tile([P, P], F32R, tag="tpr", name="tpr")
                    nc.tensor.transpose(tps, xT[c][:, tt * P:(tt + 1) * P], identR)
                    if (tt * NCT + c) % 2:
                        nc.scalar.copy(out=otok[:, c * P:(c + 1) * P], in_=tps)
                    else:
                        nc.vector.tensor_copy(out=otok[:, c * P:(c + 1) * P], in_=tps)
                nc.sync.dma_start(out=out_d.ap()[tt * P:(tt + 1) * P, :], in_=otok)
            tp_cm.__exit__(None, None, None)
            tok_cm.__exit__(None, None, None)

        for tch in range(2):
            tsl = slice(512 * tch, 512 * (tch + 1))
            h2_t = h2_both[tch]
            utiles = [up.tile([P, 2, 512], BF16, tag="u", name="u") for _ in range(16)]
            w5_cm, w5 = openp(name=f"w5{tch}", bufs=3)
            accu_cm, accu = openp(name=f"psU{tch}", bufs=2, space="PSUM")
            for f2 in range(NFT // 2):
                wsl = load_wpair(W2d("W_fc"), f2, w5)
                for fh in range(2):
                    ff = 2 * f2 + fh
                    ups = accu.tile([P, 512], F32, tag="acc", name="acc")
                    for c in range(NCT):
                        nc.tensor.matmul(ups, wsl[:, c, 128 * fh:128 * (fh + 1)], h2_t[c],
                                         start=(c == 0), stop=(c == NCT - 1))
                    nc.scalar.activation(out=utiles[ff // 2][:, ff % 2, :], in_=ups,
                                         func=AF.Gelu_apprx_tanh,
                                         bias=bfc_c[:, ff:ff + 1], scale=1.0)
                if tch == 1 and f2 == 3:
                    # transpose-out of chunk 0 overlaps chunk 1's fc matmuls
                    transpose_out(0)
            accu_cm.__exit__(None, None, None)
            w5_cm.__exit__(None, None, None)

            w6_cm, w6 = openp(name=f"w6{tch}", bufs=4)
            psM_cm, psM = openp(name=f"psM{tch}", bufs=8, space="PSUM")
            mps = [psM.tile([P, 512], F32, tag="m", name="m") for _ in range(NCT)]
            for ff in range(NFT):
                wr = w6.tile([P, C], BF16, tag="mps", name="mps")
                nc.sync.dma_start(out=wr, in_=W2d("W_mproj")[ff * P:(ff + 1) * P, :])
                for co in range(NCT):
                    nc.tensor.matmul(mps[co], wr[:, co * P:(co + 1) * P],
                                     utiles[ff // 2][:, ff % 2, :],
                                     start=(ff == 0), stop=(ff == NFT - 1))
            for co in range(NCT):
                nc.vector.scalar_tensor_tensor(
                    out=xT[co][:, tsl], in0=mps[co], scalar=bmp_c[:, co:co + 1],
                    in1=xT[co][:, tsl], op0=OP.add, op1=OP.add)
            psM_cm.__exit__(None, None, None)
            w6_cm.__exit__(None, None, None)
        transpose_out(1)

        h3_cm.__exit__(None, None, None)
        up_cm.__exit__(None, None, None)
        xT_cm.__exit__(None, None, None)

        for cm in reversed(kw_cms):
            cm.__exit__(None, None, None)

    nc.compile()
    return nc


def kernel(**inputs):
    import ml_dtypes
    from concourse.bass_utils import run_bass_kernel_spmd

    if "nc" not in _CACHED:
        _CACHED["nc"] = _build()
    nc = _CACHED["nc"]

    f32 = {k: np.asarray(v, dtype=np.float32) for k, v in inputs.items()}
    # Fold LN gains into the consuming weights and LN biases into the
    # consuming projection biases: W^T(xhat*g + b) = (W*g[:,None])^T xhat
    # + W^T b. Exact for any g/b; on-chip LN then only applies (x-mu)*rstd.
    g1, b1v = f32["ln1_g"], f32["ln1_b"]
    g2, b2v = f32["ln2_g"], f32["ln2_b"]
    W_attn, Wq, W_fc = f32["W_attn"], f32["Wq"], f32["W_fc"]
    f32 = dict(f32)
    f32["b_attn"] = f32["b_attn"] + W_attn.T @ b1v
    f32["W_attn"] = W_attn * g1[:, None]
    f32["bq"] = f32["bq"] + Wq.T @ b1v
    f32["Wq"] = Wq * g1[:, None]
    f32["b_fc"] = f32["b_fc"] + W_fc.T @ b2v
    f32["W_fc"] = W_fc * g2[:, None]
    np_inputs = {}
    for k, v in f32.items():
        np_inputs[k] = v.astype(ml_dtypes.bfloat16) if k in WEIGHT_NAMES else v
    in_maps = []
    for b in range(B):
        m = dict(np_inputs)
        m["x"] = np.ascontiguousarray(np_inputs["x"][b])
        m["x_img_feats"] = np.ascontiguousarray(np_inputs["x_img_feats"][b])
        in_maps.append(m)
    res = run_bass_kernel_spmd(nc, in_maps, core_ids=list(range(B)))
    out = np.stack([res.results[b]["out"] for b in range(B)], axis=0)
    return out.astype(np.float32)


# revision 38
# speedup vs baseline: 1.3550x; 1.0285x over previous
"""Trainium2 Bass kernel for a dense transformer block (self-attn + cross-attn + MLP).

Sharding: data-parallel over batch, one batch element per NeuronCore (B=8, 8 cores),
no collectives. All activations are kept feature-major ([C, T]) on chip so every
projection matmul consumes weight tiles directly from DRAM.

Perf structure:
- bf16 weights (host-cast) stream straight into matmul-ready SBUF tiles as
  256-column pairs (512B DRAM rows per descriptor); no on-chip cast pass.
- Matmul moving operands (LN outputs, q/k, attention probabilities, gelu
  outputs) are bf16: full PE rate at any moving width. Residual stays f32r,
  carried in-place in the xT tiles.
- Softmax: S^T layout; denominator from a ones column in the augmented V; the
  reciprocal row is partition-broadcast on gpsimd (not PE+Act).
- Causal attention trims S/AV matmuls and exp to the unmasked column span.
- Emission interleaves independent PE work into the Act-bound attention
  windows: cross-attn img transpose + K/V projections run during self-attn;
  a_proj / q-proj for token chunk 0 run during chunk 1's attention; cproj
  chunk 0 runs during cross-attn chunk 1; transpose-out of chunk 0 runs
  during the MLP's chunk 1.
- Biases load as [nf,128] row tiles (contiguous DMA) and are PE-transposed to
  per-partition columns, avoiding 4-byte-descriptor DMA storms.
"""

import sys
import numpy as np

sys.path.insert(0, "/opt/trn_rl_repo")

B, T, C = 8, 1024, 1024
H = 16
D = C // H          # 64
TI = 256
FF = 4 * C          # 4096
EPS = 1e-5
NCT = C // 128      # 8 c tiles
NTT = T // 128      # 8 t tiles
NFT = FF // 128     # 32 ff tiles
P = 128

_CACHED = {}

WEIGHT_NAMES = ("W_attn", "W_aproj", "Wq", "Wk", "Wv", "Wcproj", "W_fc", "W_mproj")


def _build():
    import concourse.tile as tile
    from concourse import bacc, mybir
    from concourse.masks import make_identity

    F32, F32R, BF16 = mybir.dt.float32, mybir.dt.float32r, mybir.dt.bfloat16
    AF = mybir.ActivationFunctionType
    OP = mybir.AluOpType

    nc = bacc.Bacc("TRN2", target_bir_lowering=False, debug=False, num_devices=8)

    dr = {}
    dr["x"] = nc.dram_tensor("x", [T, C], F32, kind="ExternalInput")
    dr["x_img_feats"] = nc.dram_tensor("x_img_feats", [TI, C], F32, kind="ExternalInput")
    for nm, shp in [
        ("ln1_g", [C]), ("ln1_b", [C]), ("ln2_g", [C]), ("ln2_b", [C]),
        ("b_attn", [3 * C]), ("b_aproj", [C]),
        ("bq", [C]), ("bk", [C]), ("bv", [C]), ("bcproj", [C]),
        ("b_fc", [FF]), ("b_mproj", [C]),
    ]:
        dr[nm] = nc.dram_tensor(nm, shp, F32, kind="ExternalInput")
    for nm, shp in [
        ("W_attn", [C, 3 * C]), ("W_aproj", [C, C]),
        ("Wq", [C, C]), ("Wk", [C, C]), ("Wv", [C, C]), ("Wcproj", [C, C]),
        ("W_fc", [C, FF]), ("W_mproj", [FF, C]),
    ]:
        dr[nm] = nc.dram_tensor(nm, shp, BF16, kind="ExternalInput")
    out_d = nc.dram_tensor("out", [T, C], F32, kind="ExternalOutput")

    def W2d(name):
        return dr[name].ap()

    with tile.TileContext(nc) as tc, nc.allow_low_precision(
        reason="bf16 weights/activations are within the 2e-2 tolerance"
    ):
        kw_cms = []

        def openp(**kw):
            cm = tc.tile_pool(**kw)
            return cm, cm.__enter__()

        def openkw(**kw):
            cm, p = openp(**kw)
            kw_cms.append(cm)
            return p

        # ---------------- kernel-wide pools (left-stack base) ----------------
        constp = openkw(name="const", bufs=1)
        fsrp = openkw(name="fsr", bufs=2)       # f32r [128,512] squares
        abp = openkw(name="ab", bufs=1)         # A_b/B_b [128,1024]
        rowp = openkw(name="rows", bufs=4)      # one [1,1024] "row" tag
        rbp = openkw(name="rb", bufs=2)         # [64,512] + [1,512] rden

        # ---------------- constants ----------------
        identB = constp.tile([P, P], BF16)
        ones_col = constp.tile([P, 16], BF16)
        nc.vector.memset(ones_col, 1.0)
        ones_f = constp.tile([P, 1], F32)
        nc.vector.memset(ones_f, 1.0)
        ones128R = constp.tile([P, 1], F32R)
        nc.vector.tensor_copy(out=ones128R, in_=ones_f)
        eps_t = constp.tile([1, 1], F32)
        nc.vector.memset(eps_t, EPS)
        eps_c = constp.tile([P, 1], F32)
        nc.vector.memset(eps_c, EPS)
        # selector: sel8[k, tt*128+j] = 1 iff k == tt (for row-broadcast matmuls)
        sel8f = constp.tile([8, NTT * P], F32)
        nc.gpsimd.memset(sel8f, 1.0)
        nc.gpsimd.affine_select(
            out=sel8f, in_=sel8f, compare_op=OP.is_ge, fill=0.0,
            base=0, pattern=[[1, NTT * P]], channel_multiplier=-P)
        nc.gpsimd.affine_select(
            out=sel8f, in_=sel8f, compare_op=OP.is_ge, fill=0.0,
            base=P - 1, pattern=[[-1, NTT * P]], channel_multiplier=P)
        sel8 = constp.tile([8, NTT * P], F32R)
        nc.vector.tensor_copy(out=sel8, in_=sel8f)

        # diagonal causal mask [128, 128]: keep (1.0) iff col >= row.
        master = constp.tile([P, P], BF16)
        nc.gpsimd.memset(master, 1.0)
        nc.gpsimd.affine_select(
            out=master, in_=master, compare_op=OP.is_ge, fill=0.0,
            base=0, pattern=[[1, P]], channel_multiplier=-1)

        # bias/gain columns: contiguous [nf,128] row DMA + one PE transpose
        psB_cm, psB = openp(name="psB", bufs=2, space="PSUM")
        stg_cm, stgp = openp(name="bstage", bufs=3)
        idf = constp.tile([P, P], F32)
        make_identity(nc, idf)
        nc.vector.tensor_copy(out=identB, in_=idf)
        identR = constp.tile([P, P], F32R)
        nc.gpsimd.tensor_copy(out=identR, in_=idf)

        def load_cols(name, nf, lo=0):
            row = stgp.tile([1, nf * P], F32, tag=name + "_s", name=name + "_s",
                            bufs=1)
            nc.sync.dma_start(
                out=row,
                in_=dr[name].ap()[lo:lo + nf * P].rearrange("(a c) -> a c", a=1))
            tps = psB.tile([P, nf], F32, tag="bt", name="bt")
            for i in range(nf):
                nc.tensor.transpose(tps[:, i:i + 1], row[0:1, i * P:(i + 1) * P],
                                    idf[0:1, 0:1])
            t = constp.tile([P, nf], F32, name=name + "_c")
            nc.vector.tensor_copy(out=t, in_=tps)
            return t

        bqk = load_cols("b_attn", 16)
        bq_c = load_cols("bq", NCT)
        bk_c = load_cols("bk", NCT)
        bap_c = load_cols("b_aproj", NCT)
        bcp_c = load_cols("bcproj", NCT)
        bmp_c = load_cols("b_mproj", NCT)
        bfc_c = load_cols("b_fc", NFT)
        stg_cm.__exit__(None, None, None)
        psB_cm.__exit__(None, None, None)

        # ---------------- helpers ----------------
        def bcast_row_bf16(row_f32, dest_pool, tag):
            rowb = rowp.tile([1, C], BF16, tag="row", name="rowb")
            nc.vector.tensor_copy(out=rowb, in_=row_f32)
            dest = dest_pool.tile([P, C], BF16, tag=tag, name=tag)
            nc.gpsimd.partition_broadcast(dest, rowb)
            return dest

        def load_wpair(wap, co2, wpool):
            """W[:, co2*256:(co2+1)*256] bf16 -> [128, K/128, 256] slab."""
            nk = wap.shape[0] // P
            wr = wpool.tile([P, nk, 256], BF16, tag="ws", name="ws")
            nc.sync.dma_start(
                out=wr,
                in_=wap[:, co2 * 256:(co2 + 1) * 256]
                .rearrange("(c p) f -> p c f", p=P))
            return wr

        def ln_stats_tch(xtiles, psp, tch, A_b, B_b):
            sl = slice(512 * tch, 512 * (tch + 1))
            sp = psp.tile([1, 512], F32, tag="lnsum", name="lnsum")
            qp = psp.tile([1, 512], F32, tag="lnsq", name="lnsq")
            for c in range(NCT):
                xs = xtiles[c][:, sl]
                nc.tensor.matmul(sp, ones128R, xs, start=(c == 0), stop=(c == NCT - 1))
                sq = fsrp.tile([P, 512], F32R, tag="sq", name="sq")
                if c % 2:
                    nc.scalar.activation(out=sq, in_=xs, func=AF.Square, scale=1.0)
                else:
                    nc.vector.tensor_tensor(out=sq, in0=xs, in1=xs, op=OP.mult)
                nc.tensor.matmul(qp, ones128R, sq, start=(c == 0), stop=(c == NCT - 1))
            mu = rowp.tile([1, 512], F32, tag="row", name="mu")
            msq = rowp.tile([1, 512], F32, tag="row", name="msq")
            nc.vector.tensor_scalar_mul(out=mu, in0=sp, scalar1=1.0 / C)
            nc.vector.tensor_scalar_mul(out=msq, in0=qp, scalar1=1.0 / C)
            musq = rowp.tile([1, 512], F32, tag="row", name="musq")
            nc.vector.tensor_tensor(out=musq, in0=mu, in1=mu, op=OP.mult)
            nc.vector.tensor_tensor(out=msq, in0=msq, in1=musq, op=OP.subtract)
            nc.scalar.activation(out=musq, in_=msq, func=AF.Sqrt, bias=eps_t, scale=1.0)
            arow = rowp.tile([1, 512], BF16, tag="row", name="arow")
            nc.vector.reciprocal(out=arow, in_=musq)
            brow = rowp.tile([1, 512], BF16, tag="row", name="brow")
            nc.vector.scalar_tensor_tensor(out=brow, in0=mu, scalar=-1.0, in1=arow,
                                           op0=OP.mult, op1=OP.mult)
            nc.gpsimd.partition_broadcast(A_b[:, sl], arow)
            nc.gpsimd.partition_broadcast(B_b[:, sl], brow)

        def ln_ab():
            A_b = abp.tile([P, T], BF16, tag="A_b", name="A_b")
            B_b = abp.tile([P, T], BF16, tag="B_b", name="B_b")
            return A_b, B_b

        def ln_apply(xtiles, A_b, B_b, hpool, tsl=slice(0, T)):
            # ht = x*A_b + B_b (LN gain/bias are folded into the consuming
            # weights host-side). Two plain tensor_tensor ops, in place in the
            # bf16 h tile, alternating gpsimd/DVE.
            w = tsl.stop - tsl.start
            htiles = []
            for c in range(NCT):
                e2 = nc.gpsimd if c % 2 == 0 else nc.vector
                ht = hpool.tile([P, w], BF16, tag="h", name="h")
                nc.vector.tensor_tensor(out=ht, in0=xtiles[c][:, tsl], in1=A_b[:, tsl],
                                        op=OP.mult)
                e2.tensor_tensor(out=ht, in0=ht, in1=B_b[:, tsl], op=OP.add)
                htiles.append(ht)
            return htiles

        def attn_chunk(kq_of, vaug_tiles, n_s, otiles, h, tch, psp, ppool, causal):
            (kt, ko), (qt, qo) = kq_of(h)
            base = 512 * tch
            sp_tiles, offs = [], []
            for st in range(n_s):
                off = max(0, P * st - base) if causal else 0
                sps = psp.tile([P, 512], F32, tag="s", name="s")
                nc.tensor.matmul(sps[:, off:512], kt[ko:ko + D, st * P:(st + 1) * P],
                                 qt[qo:qo + D, base + off:base + 512],
                                 start=True, stop=True, tile_position=(ko, 0))
                sp_tiles.append(sps)
                offs.append(off)
            ptiles = []
            for st in range(n_s):
                off = offs[st]
                pt = ppool.tile([P, 512], BF16, tag="p", name="p")
                nc.scalar.activation(out=pt[:, off:512], in_=sp_tiles[st][:, off:512],
                                     func=AF.Exp, scale=0.125)
                if causal and P * st - base >= 0:
                    nc.vector.tensor_tensor(out=pt[:, off:off + P], in0=pt[:, off:off + P],
                                            in1=master, op=OP.mult)
                ptiles.append(pt)
            ops = psp.tile([65, 512], F32, tag="o", name="o")
            for st in range(n_s):
                off = offs[st]
                nc.tensor.matmul(ops[:, off:512], vaug_tiles[st][:, 65 * h:65 * h + 65],
                                 ptiles[st][:, off:512], start=(st == 0),
                                 stop=(st == n_s - 1), skip_group_check=True)
            rden = rbp.tile([1, 512], BF16, tag="rden", name="rden")
            nc.vector.reciprocal(out=rden, in_=ops[64:65, :])
            rb = rbp.tile([64, 512], BF16, tag="rb", name="rb")
            nc.gpsimd.partition_broadcast(rb, rden)
            ot = otiles[h // 2]
            po = (h % 2) * D
            nc.vector.tensor_tensor(
                out=ot[po:po + D, base:base + 512],
                in0=ops[0:64, :], in1=rb, op=OP.mult)

        # ===== psLN psum pool: LN statistics banks, lives until the MLP =====
        psLN_cm, psLN = openp(name="psLN", bufs=1, space="PSUM")

        # ========== right stack: xT doubles as the residual stream ==========
        xT_cm, xTp = openp(name="xT", bufs=NCT, side="right")
        xT = [xTp.tile([P, T], F32R, tag="xT", name="xT") for _ in range(NCT)]

        # imgT lives (left) from P0 until cross attention ends
        img_cm, imgp = openp(name="img", bufs=NCT)
        imgT = [imgp.tile([P, TI], BF16, tag="imgT", name="imgT") for _ in range(NCT)]

        # ================= P0: load & transpose x and img =================
        h1_cm, hp = openp(name="h1", bufs=2 * NCT)
        bn_cm, bnp = openp(name="bn0", bufs=3)
        tok_cm, tokp = openp(name="tok0", bufs=4)
        tp_cm, tpp = openp(name="psT0", bufs=4, space="PSUM")
        tpx_cm, tpxp = openp(name="psTx", bufs=2, space="PSUM")
        cpeng = [nc.vector, nc.scalar, nc.gpsimd]

        def transpose_tok(src_ap, dst_tiles, tt, dst_off, mv=None):
            tok = tokp.tile([P, C], F32, tag="tok", name="tok")
            nc.sync.dma_start(out=tok, in_=src_ap[tt * P:(tt + 1) * P, :])
            if mv is not None:
                bn6 = bnp.tile([P, 12], F32, tag="bn6", name="bn6")
                nc.vector.bn_stats(bn6[:, 0:6], tok[:, 0:512])
                nc.vector.bn_stats(bn6[:, 6:12], tok[:, 512:1024])
                nc.vector.bn_aggr(mv[:, tt, :], bn6)
            for c in range(NCT):
                tps = tpp.tile([P, P], F32, tag="tp", name="tp")
                nc.tensor.transpose(tps, tok[:, c * P:(c + 1) * P], idf)
                sl = slice(dst_off + tt * P, dst_off + (tt + 1) * P)
                if (tt * NCT + c) % 3 != 0:
                    nc.scalar.copy(out=dst_tiles[c][:, sl], in_=tps)
                else:
                    nc.vector.tensor_copy(out=dst_tiles[c][:, sl], in_=tps)

        mv = bnp.tile([P, NTT, 2], F32, tag="mv", name="mv")
        for tt in range(NTT):
            transpose_tok(dr["x"].ap(), xT, tt, 0, mv=mv)

        # token-major LN1 stats -> feature-major A_b/B_b broadcast tiles
        A_b, B_b = ln_ab()
        sd8 = bnp.tile([P, NTT], F32, tag="sd8", name="sd8")
        nc.scalar.activation(out=sd8, in_=mv[:, :, 1], func=AF.Sqrt,
                             bias=eps_c, scale=1.0)
        ar8 = bnp.tile([P, NTT], F32R, tag="ar8", name="ar8")
        nc.vector.reciprocal(out=ar8, in_=sd8)
        br8 = bnp.tile([P, NTT], F32R, tag="br8", name="br8")
        nc.vector.scalar_tensor_tensor(out=br8, in0=mv[:, :, 0], scalar=-1.0,
                                       in1=ar8, op0=OP.mult, op1=OP.mult)
        abT = [bnp.tile([NTT, P], F32R, tag=f"abT{i}", name="abT") for i in range(2)]
        for i, r8 in enumerate((ar8, br8)):
            tpx = tpxp.tile([NTT, P], F32R, tag="tpx", name="tpx")
            nc.tensor.transpose(tpx, r8, identR)
            nc.vector.tensor_copy(out=abT[i], in_=tpx)
        for i, dst in enumerate((A_b, B_b)):
            for tt in range(NTT):
                bps = tpp.tile([P, P], F32, tag="tp", name="tp")
                nc.tensor.matmul(bps, sel8[:, tt * P:(tt + 1) * P], abT[i],
                                 start=True, stop=True)
                eng = nc.scalar if (i * NTT + tt) % 2 else nc.vector
                if eng is nc.scalar:
                    nc.scalar.copy(out=dst[:, tt * P:(tt + 1) * P], in_=bps)
                else:
                    nc.vector.tensor_copy(out=dst[:, tt * P:(tt + 1) * P], in_=bps)

        h_t = [[None] * NCT, [None] * NCT]
        for tch in range(2):
            h_t[tch][:] = ln_apply(xT, A_b, B_b, hp,
                                   tsl=slice(512 * tch, 512 * (tch + 1)))
        for tt in range(TI // P):
            transpose_tok(dr["x_img_feats"].ap(), imgT, tt, 0)
        tpx_cm.__exit__(None, None, None)
        tp_cm.__exit__(None, None, None)
        tok_cm.__exit__(None, None, None)
        bn_cm.__exit__(None, None, None)

        # ====== merged qkv + self-attention (+ hoisted cross-attn prep) ======
        vap_cm, vap = openp(name="vaug", bufs=NTT, side="right")
        vaug = [vap.tile([P, 16 * 65], BF16, tag="va", name="va") for _ in range(NTT)]
        qk_cm, qkp = openp(name="qk", bufs=16, side="right")
        qk_t = [qkp.tile([P, T], BF16, tag="qk", name="qk") for _ in range(16)]

        k2_cm, k2p = openp(name="k2", bufs=NCT)
        v2_cm, v2p = openp(name="v2", bufs=2)
        hb_cm, hbp = openp(name="hb", bufs=NCT)
        q2_cm, q2p = openp(name="q2", bufs=NCT)
        w23_cm, w23 = openp(name="w23", bufs=2)
        psAC_cm, accp = openp(name="psAC", bufs=2, space="PSUM")

        o_cm, opool = openp(name="o1", bufs=NCT)
        pp_cm, pp = openp(name="pp1", bufs=6)
        psS_cm, psS = openp(name="psS1", bufs=2, space="PSUM")

        otiles = [opool.tile([P, T], BF16, tag="ot", name="ot") for _ in range(NCT)]
        k2_t = [k2p.tile([P, TI], BF16, tag="k2", name="k2") for _ in range(NCT)]
        v2aug = [v2p.tile([P, 16 * 65], BF16, tag="va2", name="va2")
                 for _ in range(TI // P)]
        q2_t = [q2p.tile([P, T], BF16, tag="q2", name="q2") for _ in range(NCT)]

        def kq_self(h):
            return (qk_t[8 + h // 2], (h % 2) * D), (qk_t[h // 2], (h % 2) * D)

        def kq_cross(h):
            return (k2_t[h // 2], (h % 2) * D), (q2_t[h // 2], (h % 2) * D)

        side = []

        def drain(n=1):
            for _ in range(n):
                if side:
                    side.pop(0)()

        brow_v = rowp.tile([1, C], F32, tag="row", name="braw")
        nc.sync.dma_start(out=brow_v,
                          in_=dr["b_attn"].ap()[2 * C:3 * C].rearrange("(a c) -> a c", a=1))
        bvb1 = bcast_row_bf16(brow_v, w23, "bvb")
        brow_v2 = rowp.tile([1, C], F32, tag="row", name="braw2")
        nc.sync.dma_start(out=brow_v2, in_=dr["bv"].ap().rearrange("(a c) -> a c", a=1))
        bvb2 = bcast_row_bf16(brow_v2, w23, "bvb2")
        for tt in range(NTT):
            nc.gpsimd.tensor_copy(
                out=vaug[tt].rearrange("p (h x) -> p h x", x=65)[:, :, 64:65],
                in_=ones_col.rearrange("p (h x) -> p h x", x=1))

        def v_group(cc, vtch):
            wr = load_wpair(W2d("W_attn"), 8 + cc, w23)
            for tt in range(4 * vtch, 4 * vtch + 4):
                vps = accp.tile([P, 512], F32, tag="acc", name="acc")[:, 0:256]
                for c in range(NCT):
                    nc.tensor.matmul(
                        vps, h_t[tt // 4][c][:, (tt % 4) * P:(tt % 4 + 1) * P],
                        wr[:, c, :], start=(c == 0), stop=(c == NCT - 1))
                dst = vaug[tt].rearrange("p (h x) -> p h x", x=65)[:, 4 * cc:4 * (cc + 1), 0:64]
                nc.vector.tensor_tensor(
                    out=dst, in0=vps.rearrange("p (h x) -> p h x", x=64),
                    in1=bvb1[:, 256 * cc:256 * (cc + 1)].rearrange("p (h x) -> p h x", x=64),
                    op=OP.add)

        def qk_pair(p_idx, dst0, bias0, tch):
            wsl = load_wpair(W2d("W_attn"), p_idx, w23)
            for fh in range(2):
                qt = qk_t[dst0 + fh]
                aps = accp.tile([P, 512], F32, tag="acc", name="acc")
                for c in range(NCT):
                    nc.tensor.matmul(aps, wsl[:, c, 128 * fh:128 * (fh + 1)],
                                     h_t[tch][c],
                                     start=(c == 0), stop=(c == NCT - 1))
                nc.scalar.activation(out=qt[:, 512 * tch:512 * (tch + 1)], in_=aps,
                                     func=AF.Identity,
                                     bias=bqk[:, bias0 + fh:bias0 + fh + 1], scale=1.0)

        def k2_group(f2):
            def go():
                wsl = load_wpair(W2d("Wk"), f2, w23)
                for fh in range(2):
                    f = 2 * f2 + fh
                    aps = accp.tile([P, 512], F32, tag="acc", name="acc")
                    for c in range(NCT):
                        nc.tensor.matmul(aps[:, 0:TI], wsl[:, c, 128 * fh:128 * (fh + 1)],
                                         imgT[c], start=(c == 0), stop=(c == NCT - 1))
                    nc.vector.tensor_scalar(out=k2_t[f], in0=aps[:, 0:TI],
                                            scalar1=bk_c[:, f:f + 1], scalar2=None,
                                            op0=OP.add)
            return go

        def v2_group(cc):
            def go():
                wr = load_wpair(W2d("Wv"), cc, w23)
                for st in range(TI // P):
                    vps = accp.tile([P, 512], F32, tag="acc", name="acc")[:, 0:256]
                    for c in range(NCT):
                        nc.tensor.matmul(vps, imgT[c][:, st * P:(st + 1) * P], wr[:, c, :],
                                         start=(c == 0), stop=(c == NCT - 1))
                    dst = v2aug[st].rearrange("p (h x) -> p h x", x=65)[:, 4 * cc:4 * (cc + 1), 0:64]
                    nc.vector.tensor_tensor(
                        out=dst, in0=vps.rearrange("p (h x) -> p h x", x=64),
                        in1=bvb2[:, 256 * cc:256 * (cc + 1)].rearrange("p (h x) -> p h x", x=64),
                        op=OP.add)
            return go

        def v2_ones():
            for st in range(TI // P):
                nc.gpsimd.tensor_copy(
                    out=v2aug[st].rearrange("p (h x) -> p h x", x=65)[:, :, 64:65],
                    in_=ones_col.rearrange("p (h x) -> p h x", x=1))

        # ---- post-attention pipeline: aproj, ln1b, q2, cross attention ----
        def aproj_co2(co2, tch):
            def go():
                sl = slice(512 * tch, 512 * (tch + 1))
                wsl = load_wpair(W2d("W_aproj"), co2, w23)
                for ch in range(2):
                    co = 2 * co2 + ch
                    aps = accp.tile([P, 512], F32, tag="acc", name="acc")
                    for c in range(NCT):
                        nc.tensor.matmul(aps, wsl[:, c, 128 * ch:128 * (ch + 1)],
                                         otiles[c][:, sl],
                                         start=(c == 0), stop=(c == NCT - 1))
                    nc.vector.scalar_tensor_tensor(
                        out=xT[co][:, sl], in0=aps, scalar=bap_c[:, co:co + 1],
                        in1=xT[co][:, sl], op0=OP.add, op1=OP.add)
            return go

        A_b2, B_b2 = ln_ab()

        def ln1b_tch(tch):
            def go():
                ln_stats_tch(xT, psLN, tch, A_b2, B_b2)
            return go

        hb_t = [[None] * NCT for _ in range(2)]

        def hb_tch(tch):
            def go():
                hb_t[tch][:] = ln_apply(
                    xT, A_b2, B_b2, hbp,
                    tsl=slice(512 * tch, 512 * (tch + 1)))
            return go

        def q2_group(f2, tch):
            def go():
                wsl = load_wpair(W2d("Wq"), f2, w23)
                for fh in range(2):
                    f = 2 * f2 + fh
                    aps = accp.tile([P, 512], F32, tag="acc", name="acc")
                    for c in range(NCT):
                        nc.tensor.matmul(aps, wsl[:, c, 128 * fh:128 * (fh + 1)],
                                         hb_t[tch][c],
                                         start=(c == 0), stop=(c == NCT - 1))
                    nc.scalar.activation(
                        out=q2_t[f][:, 512 * tch:512 * (tch + 1)], in_=aps,
                        func=AF.Identity, bias=bq_c[:, f:f + 1], scale=1.0)
            return go

        side += [k2_group(f2) for f2 in range(4)]
        side += [v2_group(cc) for cc in range(4)]
        side.append(v2_ones)

        for tch in range(2):
            if tch == 1:
                side += [aproj_co2(co2, 0) for co2 in range(4)]
                side += [ln1b_tch(0), hb_tch(0)]
                side += [q2_group(f2, 0) for f2 in range(4)]
            for g in range(4):
                v_group(g, tch)
                qk_pair(g, 2 * g, 2 * g, tch)
                qk_pair(4 + g, 8 + 2 * g, 8 + 2 * g, tch)
                for h in range(4 * g, 4 * g + 4):
                    attn_chunk(kq_self, vaug, 4 * (tch + 1), otiles, h, tch, psS, pp,
                               causal=True)
                    drain(1)
        drain(len(side))
        qk_cm.__exit__(None, None, None)
        vap_cm.__exit__(None, None, None)

        # ---- cross attention (q2 written post-attention) ----
        psS2 = psS
        pp2 = pp
        o2_cm, opool2 = openp(name="o2", bufs=NCT)
        o2tiles = [opool2.tile([P, T], BF16, tag="ot", name="ot") for _ in range(NCT)]

        def cproj_co2(co2, tch):
            def go():
                sl = slice(512 * tch, 512 * (tch + 1))
                wsl = load_wpair(W2d("Wcproj"), co2, w23)
                for ch in range(2):
                    co = 2 * co2 + ch
                    aps = accp.tile([P, 512], F32, tag="acc", name="acc")
                    for c in range(NCT):
                        nc.tensor.matmul(aps, wsl[:, c, 128 * ch:128 * (ch + 1)],
                                         o2tiles[c][:, sl],
                                         start=(c == 0), stop=(c == NCT - 1))
                    nc.vector.scalar_tensor_tensor(
                        out=xT[co][:, sl], in0=aps, scalar=bcp_c[:, co:co + 1],
                        in1=xT[co][:, sl], op0=OP.add, op1=OP.add)
            return go

        side = [aproj_co2(co2, 1) for co2 in range(4)]
        side += [ln1b_tch(1), hb_tch(1)]
        side += [q2_group(f2, 1) for f2 in range(4)]
        for h in range(H):
            attn_chunk(kq_cross, v2aug, TI // P, o2tiles, h, 0, psS2, pp2,
                       causal=False)
            drain(1)
        drain(len(side))
        side = [cproj_co2(co2, 0) for co2 in range(4)]
        for h in range(H):
            attn_chunk(kq_cross, v2aug, TI // P, o2tiles, h, 1, psS2, pp2,
                       causal=False)
            drain(1)
        drain(len(side))
        for co2 in range(4):
            cproj_co2(co2, 1)()

        o2_cm.__exit__(None, None, None)
        psS_cm.__exit__(None, None, None)
        pp_cm.__exit__(None, None, None)
        o_cm.__exit__(None, None, None)
        psAC_cm.__exit__(None, None, None)
        w23_cm.__exit__(None, None, None)
        q2_cm.__exit__(None, None, None)
        hb_cm.__exit__(None, None, None)
        v2_cm.__exit__(None, None, None)
        k2_cm.__exit__(None, None, None)
        h1_cm.__exit__(None, None, None)
        img_cm.__exit__(None, None, None)

        # ================= P6: MLP =================
        A_b3, B_b3 = ln_ab()
        ln_stats_tch(xT, psLN, 0, A_b3, B_b3)
        ln_stats_tch(xT, psLN, 1, A_b3, B_b3)
        psLN_cm.__exit__(None, None, None)

        up_cm, up = openp(name="u", bufs=16, side="right")
        h3_cm, h3p = openp(name="h3", bufs=2 * NCT)
        h2_both = [ln_apply(xT, A_b3, B_b3, h3p,
                            tsl=slice(512 * t, 512 * (t + 1))) for t in range(2)]

        def transpose_out(tch):
            tok_cm, tokp = openp(name=f"tok7{tch}", bufs=2)
            tp_cm, tpp = openp(name=f"psT7{tch}", bufs=4, space="PSUM")
            for tt in range(4 * tch, 4 * (tch + 1)):
                otok = tokp.tile([P, C], F32, tag="tok", name="tok")
                for c in range(NCT):
                    tps = tpp.tile([P, P], F32R, tag="tpr", name="tpr")
                    nc.tensor.transpose(tps, xT[c][:, tt * P:(tt + 1) * P], identR)
                    if (tt * NCT + c) % 2:
                        nc.scalar.copy(out=otok[:, c * P:(c + 1) * P], in_=tps)
                    else:
                        nc.vector.tensor_copy(out=otok[:, c * P:(c + 1) * P], in_=tps)
                nc.sync.dma_start(out=out_d.ap()[tt * P:(tt + 1) * P, :], in_=otok)
            tp_cm.__exit__(None, None, None)
            tok_cm.__exit__(None, None, None)

        for tch in range(2):
            tsl = slice(512 * tch, 512 * (tch + 1))
            h2_t = h2_both[tch]
            utiles = [up.tile([P, 2, 512], BF16, tag="u", name="u") for _ in range(16)]
            w5_cm, w5 = openp(name=f"w5{tch}", bufs=3)
            accu_cm, accu = openp(name=f"psU{tch}", bufs=2, space="PSUM")
            for f2 in range(NFT // 2):
                wsl = load_wpair(W2d("W_fc"), f2, w5)
                for fh in range(2):
                    ff = 2 * f2 + fh
                    ups = accu.tile([P, 512], F32, tag="acc", name="acc")
                    for c in range(NCT):
                        nc.tensor.matmul(ups, wsl[:, c, 128 * fh:128 * (fh + 1)], h2_t[c],
                                         start=(c == 0), stop=(c == NCT - 1))
                    nc.scalar.activation(out=utiles[ff // 2][:, ff % 2, :], in_=ups,
                                         func=AF.Gelu_apprx_tanh,
                                         bias=bfc_c[:, ff:ff + 1], scale=1.0)
                if tch == 1 and f2 == 3:
                    # transpose-out of chunk 0 overlaps chunk 1's fc matmuls
                    transpose_out(0)
            accu_cm.__exit__(None, None, None)
            w5_cm.__exit__(None, None, None)

            w6_cm, w6 = openp(name=f"w6{tch}", bufs=4)
            psM_cm, psM = openp(name=f"psM{tch}", bufs=8, space="PSUM")
            mps = [psM.tile([P, 512], F32, tag="m", name="m") for _ in range(NCT)]
            for ff in range(NFT):
                wr = w6.tile([P, C], BF16, tag="mps", name="mps")
                nc.sync.dma_start(out=wr, in_=W2d("W_mproj")[ff * P:(ff + 1) * P, :])
                for co in range(NCT):
                    nc.tensor.matmul(mps[co], wr[:, co * P:(co + 1) * P],
                                     utiles[ff // 2][:, ff % 2, :],
                                     start=(ff == 0), stop=(ff == NFT - 1))
            for co in range(NCT):
                nc.vector.scalar_tensor_tensor(
                    out=xT[co][:, tsl], in0=mps[co], scalar=bmp_c[:, co:co + 1],
                    in1=xT[co][:, tsl], op0=OP.add, op1=OP.add)
            psM_cm.__exit__(None, None, None)
            w6_cm.__exit__(None, None, None)
        transpose_out(1)

        h3_cm.__exit__(None, None, None)
        up_cm.__exit__(None, None, None)
        xT_cm.__exit__(None, None, None)

        for cm in reversed(kw_cms):
            cm.__exit__(None, None, None)

    nc.compile()
    return nc


def kernel(**inputs):
    import ml_dtypes
    from concourse.bass_utils import run_bass_kernel_spmd

    if "nc" not in _CACHED:
        _CACHED["nc"] = _build()
    nc = _CACHED["nc"]

    f32 = {k: np.asarray(v, dtype=np.float32) for k, v in inputs.items()}
    # Fold LN gains into the consuming weights and LN biases into the
    # consuming projection biases: W^T(xhat*g + b) = (W*g[:,None])^T xhat
    # + W^T b. Exact for any g/b; on-chip LN then only applies (x-mu)*rstd.
    g1, b1v = f32["ln1_g"], f32["ln1_b"]
    g2, b2v = f32["ln2_g"], f32["ln2_b"]
    W_attn, Wq, W_fc = f32["W_attn"], f32["Wq"], f32["W_fc"]
    f32 = dict(f32)
    f32["b_attn"] = f32["b_attn"] + W_attn.T @ b1v
    f32["W_attn"] = W_attn * g1[:, None]
    f32["bq"] = f32["bq"] + Wq.T @ b1v
    f32["Wq"] = Wq * g1[:, None]
    f32["b_fc"] = f32["b_fc"] + W_fc.T @ b2v
    f32["W_fc"] = W_fc * g2[:, None]
    np_inputs = {}
    for k, v in f32.items():
        np_inputs[k] = v.astype(ml_dtypes.bfloat16) if k in WEIGHT_NAMES else v
    in_maps = []
    for b in range(B):
        m = dict(np_inputs)
        m["x"] = np.ascontiguousarray(np_inputs["x"][b])
        m["x_img_feats"] = np.ascontiguousarray(np_inputs["x_img_feats"][b])
        in_maps.append(m)
    res = run_bass_kernel_spmd(nc, in_maps, core_ids=list(range(B)))
    out = np.stack([res.results[b]["out"] for b in range(B)], axis=0)
    return out.astype(np.float32)
